# revision 51
# baseline (speedup 1.0000x reference)
"""MultiHeadedAttention block (B=4, S=2048, D=1024, H=16) on 8 TRN2 cores.

Sharding: core c handles batch b=c//2 and query-row half c%2 (1024 rows).
Each core computes full K/V projections for its batch (2x redundant within a
batch pair), attention for all 16 heads over its 1024 query rows, then
O-projection + residual + LayerNorm. No collectives.

All four projections and the QK^T scores run in fp8e4m3 with the DoubleRow
perf mode (2x PE rate). Scale management: weights are stored as W.T*32 in
fp8, activations x in fp8, so Q'=K'=32(xW+b) (stored fp8 in a split-dk
[32p, 2, .] layout for DoubleRow scores), V'=32(xWv+bv) (bf16). Raw scores
are 1024*(QK^T); the softmax exp applies scale 1/8192 = 1/(1024*sqrt(64)).
The ones-column of V gives the softmax denominator D via the PV matmul; the
reciprocal is broadcast with a ones(=2.0) stationary vector so
xo = pv * (2/D) = 64*(attn_out + bv) in fp8, and phase E folds the
1/(64*32) into a 1/2048 multiply before the residual add + LayerNorm.

The softmax exp is split across three engines: exact Exp on the Activation
engine plus the int16-bitcast approximation exp(x) ~= bitcast_bf16(
int16(x*128*log2e + 127*128 - 5.5)) on Pool and DVE (max ~3% weight error,
negligible through the diffuse softmax at this tolerance).
"""

import sys

if "/opt/trn_rl_repo" not in sys.path:
    sys.path.insert(0, "/opt/trn_rl_repo")

import ml_dtypes
import numpy as np

import concourse.bass as bass
import concourse.mybir as mybir
import concourse.tile as tile
from concourse.bass_utils import run_bass_kernel_spmd

B, S, D, H, DK = 4, 2048, 1024, 16, 64
P = 128
M = S // 2          # query rows per core
NDT = D // P        # 8 contraction chunks of 128
NOT = D // P        # 8 output-feature chunks (= head pairs)
NHP = H // 2        # 8 head pairs
NKT = S // P        # 16 key chunks of 128
NQT = M // 512      # 2 query 512-chunks
NRT_K = S // 512    # 4 key-row 512-chunks
NRT_V = S // P      # 16 V row chunks
NRT_O = M // P      # 8 output row chunks
KG = 2              # k-chunks per exp group
NKG = NKT // KG     # 8 exp groups per (head, qt)
F32 = mybir.dt.float32
F8 = mybir.dt.float8e4
BF16 = mybir.dt.bfloat16
I16 = mybir.dt.int16
MM_DT = mybir.dt.float32r
AF = mybir.ActivationFunctionType
ALU = mybir.AluOpType
DR = mybir.MatmulPerfMode.DoubleRow

LOG2E = 1.4426950408889634
EXP_SCALE = 1.0 / 8192.0                 # 1/(32*32*sqrt(DK))
EXP_A = 128.0 * LOG2E * EXP_SCALE        # int16-bitcast exp multiplier
EXP_B = 127.0 * 128.0 - 5.5              # exponent bias - mean sawtooth corr


def _split_sync_waits(nc, max_waits=1):
    """Split instructions carrying more than max_waits sem waits.

    The container's walrus rejects instructions with multiple sync wait
    commands, so excess waits move onto NoOp instructions inserted just
    before, on the same engine.
    """
    idx = 0
    for f in nc.m.functions:
        for blk in f.blocks:
            newl = []
            for inst in blk.instructions:
                si = inst.sync_info
                waits = list(si.on_wait) if si is not None and si.on_wait else []
                if len(waits) > max_waits:
                    extra = waits[max_waits:]
                    si.on_wait = waits[:max_waits]
                    for j in range(0, len(extra), max_waits):
                        nop = mybir.InstNoOp(name=f"I-wsplit-{idx}", ins=[], outs=[])
                        idx += 1
                        nop.engine = inst.engine
                        nop.sync_info = mybir.SyncInfo(
                            on_wait=extra[j : j + max_waits], on_update=[]
                        )
                        newl.append(nop)
                newl.append(inst)
            blk.instructions = newl


def build_nc(loops=0):
    nc = bass.Bass()
    xq8 = nc.dram_tensor("xq8", [DK, NDT, 2, M], F8, kind="ExternalInput")
    xk8 = nc.dram_tensor("xk8", [DK, NDT, 2, S], F8, kind="ExternalInput")
    xv8 = nc.dram_tensor("xv8", [DK, NDT, 2, S], F8, kind="ExternalInput")
    wq8 = nc.dram_tensor("wq8", [DK, NDT, 2, D], F8, kind="ExternalInput")
    wk8 = nc.dram_tensor("wk8", [DK, NDT, 2, D], F8, kind="ExternalInput")
    wv8 = nc.dram_tensor("wv8", [DK, NDT, 2, D], F8, kind="ExternalInput")
    wo8 = nc.dram_tensor("wo8", [DK, NDT, 2, D], F8, kind="ExternalInput")
    bq32 = nc.dram_tensor("bq32", [P, NOT], F32, kind="ExternalInput")
    bk32 = nc.dram_tensor("bk32", [P, NOT], F32, kind="ExternalInput")
    bkr = nc.dram_tensor("bkr", [NHP, P], BF16, kind="ExternalInput")
    bvb = nc.dram_tensor("bvb", [D], BF16, kind="ExternalInput")
    qres = nc.dram_tensor("qres", [M, D], BF16, kind="ExternalInput")
    gv = nc.dram_tensor("ln_g", [D], F32, kind="ExternalInput")
    bv2 = nc.dram_tensor("ln_b", [D], F32, kind="ExternalInput")
    out = nc.dram_tensor("out", [M, D], F32, kind="ExternalOutput")

    import contextlib

    with tile.TileContext(nc) as tc:
        loop_cm = tc.For_i(0, loops, 1) if loops else contextlib.nullcontext()
        loop_cm.__enter__()
        pxo_cm = tc.tile_pool(name="pxo", bufs=1)
        pxo = pxo_cm.__enter__()
        with (
            tc.tile_pool(name="pqv", bufs=1) as pqv,
        ):
            # attention outputs, fp8 split-head-pair layout for O-proj DR
            XO = [
                pxo.tile([DK, 2, M], F8, tag=f"XO{i}", name=f"XO{i}")
                for i in range(NHP)
            ]

            # Q' fp8, 2 heads across partitions: head h at partition
            # (h%2)*64, slot h//2, sub i = dk//32
            QT = pqv.tile([P, 8, 2, M], F8, tag="QT", name="QT")
            bq_p = pqv.tile([P, NOT], F32)
            bkr_t = pqv.tile([1, NHP, P], BF16)
            ones_bf = pqv.tile([1, 512], BF16)
            nc.vector.memset(ones_bf, 1.0)
            nc.sync.dma_start(bkr_t, bkr[:, :].rearrange("a p -> (a p)").partition_broadcast(1))
            bk_p = pqv.tile([P, NOT], F32)
            bv_b = pqv.tile([P, D], BF16)

            # Vt: [P, 2, H, DK] fp8; sub 1 is a 0.5-constant block so the
            # PV matmul replicates den/2 across output partitions 64..127
            Vt = []
            for rt in range(NRT_V):
                t = pqv.tile([P, H, 2, DK], F8, tag=f"Vt{rt}", name=f"Vt{rt}")
                nc.vector.memset(t[:, :, 1, :], 0.5)
                Vt.append(t)

            # wv/xv load early so phase B starts without a DMA stall
            pwv_cm = tc.tile_pool(name="pwv", bufs=NDT // 2, side="right")
            pwv = pwv_cm.__enter__()
            wvp = [
                pwv.tile([DK, 2, 2, D], F8, tag="wv", name=f"wv{dp}")
                for dp in range(NDT // 2)
            ]
            wv = [wvp[dt // 2][:, dt % 2] for dt in range(NDT)]

            pbx_cm = tc.tile_pool(name="pbx", bufs=NDT // 2, side="right")
            pbx = pbx_cm.__enter__()
            xvp = [
                pbx.tile([DK, 2, 2, S], F8, tag="xv", name=f"xv{dp}")
                for dp in range(NDT // 2)
            ]
            xv = [xvp[dt // 2][:, dt % 2] for dt in range(NDT)]

            psAB_cm = tc.tile_pool(name="psAB", bufs=8, space="PSUM")
            psAB = psAB_cm.__enter__()
            psg_cm = tc.tile_pool(name="psg", bufs=3)
            psg = psg_cm.__enter__()

            # ---- Phase A: Q' = 32*(Wq @ x_q^T) + 32 bq, fp8 DR
            with (
                tc.tile_pool(name="pa", bufs=NDT // 2) as pa,
            ):
                wqp = []
                xqp = []
                for dp in range(NDT // 2):
                    wt = pa.tile([DK, 2, 2, D], F8, tag="wq", name=f"wq{dp}")
                    nc.sync.dma_start(wt, wq8[:, 2 * dp : 2 * dp + 2, :, :])
                    wqp.append(wt)
                    xt = pa.tile([DK, 2, 2, M], F8, tag="xq", name=f"xq{dp}")
                    nc.sync.dma_start(xt, xq8[:, 2 * dp : 2 * dp + 2, :, :])
                    xqp.append(xt)
                wq = [wqp[dt // 2][:, dt % 2] for dt in range(NDT)]
                xq = [xqp[dt // 2][:, dt % 2] for dt in range(NDT)]
                # behind phase A's own loads in the HWDGE queue: biases,
                # then the wv/xv pairs phase B consumes in dt order
                nc.sync.dma_start(bq_p, bq32[:, :])
                nc.sync.dma_start(bk_p, bk32[:, :])
                nc.sync.dma_start(bv_b, bvb[:].partition_broadcast(P))
                for dp in range(NDT // 2):
                    nc.sync.dma_start(wvp[dp], wv8[:, 2 * dp : 2 * dp + 2, :, :])
                    nc.sync.dma_start(xvp[dp], xv8[:, 2 * dp : 2 * dp + 2, :, :])
                for ot in range(NOT):
                    stage = psg.tile([P, M], F8, tag="qstg", name="qstg")
                    for qt in range(NQT):
                        ps = psAB.tile([P, 512], F32, tag="ps", name="ps")
                        for dt in range(NDT):
                            nc.tensor.matmul(
                                ps,
                                wq[dt][:, :, ot * P : (ot + 1) * P],
                                xq[dt][:, :, qt * 512 : (qt + 1) * 512],
                                start=(dt == 0),
                                stop=(dt == NDT - 1),
                                perf_mode=DR,
                            )
                        nc.vector.tensor_scalar_add(
                            stage[:, qt * 512 : (qt + 1) * 512],
                            ps,
                            bq_p[:, ot : ot + 1],
                        )
                    # repack the whole ot row into the split-dk
                    # 2-heads-across-partitions layout via 4 DMAs
                    for h01 in range(2):
                        h = 2 * ot + h01
                        for i in range(2):
                            pb = h01 * DK + i * 32
                            (nc.sync if i == 0 else nc.gpsimd).dma_start(
                                QT[(h % 2) * 64 : (h % 2) * 64 + 32, h // 2, i, :],
                                stage[pb : pb + 32, :],
                            )

            # xk/wk load during phase B so phase D starts without a DMA stall
            pdx_cm = tc.tile_pool(name="pdx", bufs=NDT // 2)
            pdx = pdx_cm.__enter__()
            xkp = []
            wkp = []
            for dp in range(NDT // 2):
                xt = pdx.tile([DK, 2, 2, S], F8, tag="xk", name=f"xk{dp}")
                nc.sync.dma_start(xt, xk8[:, 2 * dp : 2 * dp + 2, :, :])
                xkp.append(xt)
                wt = pdx.tile([DK, 2, 2, D], F8, tag="wk", name=f"wk{dp}")
                nc.sync.dma_start(wt, wk8[:, 2 * dp : 2 * dp + 2, :, :])
                wkp.append(wt)
            xk = [xkp[dt // 2][:, dt % 2] for dt in range(NDT)]
            wk = [wkp[dt // 2][:, dt % 2] for dt in range(NDT)]

            # ---- Phase B: V' = 32*(x_v @ Wv^T + bv), fp8 DR, bf16 out
            for rt in range(NRT_V):
                for o2 in range(2):
                    ps = psAB.tile([P, 512], F32, tag="ps", name="ps")
                    for dt in range(NDT):
                        nc.tensor.matmul(
                            ps,
                            xv[dt][:, :, rt * P : (rt + 1) * P],
                            wv[dt][:, :, o2 * 512 : (o2 + 1) * 512],
                            start=(dt == 0),
                            stop=(dt == NDT - 1),
                            perf_mode=DR,
                        )
                    nc.vector.tensor_tensor(
                        Vt[rt][:, o2 * 8 : (o2 + 1) * 8, 0, :],
                        ps[:, :].rearrange("p (h e) -> p h e", e=DK),
                        bv_b[:, o2 * 512 : (o2 + 1) * 512].rearrange(
                            "p (h e) -> p h e", e=DK
                        ),
                        op=ALU.add,
                    )

            pbx_cm.__exit__(None, None, None)
            pwv_cm.__exit__(None, None, None)
            psAB_cm.__exit__(None, None, None)

            # wo prefetch during D so phase E starts without a DMA stall
            pwo_cm = tc.tile_pool(name="pwo", bufs=NDT, side="right")
            pwo = pwo_cm.__enter__()
            wo = []
            for dt in range(NDT):
                t = pwo.tile([DK, 2, D], F8, tag="wo", name=f"wo{dt}")
                nc.sync.dma_start(t, wo8[:, dt, :, :])
                wo.append(t)
            pec_cm = tc.tile_pool(name="pec", bufs=1, side="right")
            pec = pec_cm.__enter__()
            g_b = pec.tile([P, D], F32)
            b_b = pec.tile([P, D], F32)
            eps_t = pec.tile([P, 1], F32)
            nc.sync.dma_start(g_b, gv[:].partition_broadcast(P))
            nc.sync.dma_start(b_b, bv2[:].partition_broadcast(P))
            nc.vector.memset(eps_t, 1e-5)

            # ---- Phase D: K' projection fused with attention
            with (
                tc.tile_pool(name="pdkt", bufs=1) as pdkt,
                tc.tile_pool(name="pde", bufs=6) as pde,
                tc.tile_pool(name="pdr", bufs=4) as pdr,
                tc.tile_pool(name="psS", bufs=3, space="PSUM") as psS,
                tc.tile_pool(name="psPV", bufs=2, space="PSUM") as psPV,
            ):
                # K' fp8, same 2-heads-across-partitions layout as QT
                KT = pdkt.tile([P, 8, 2, S], F8, tag="KT", name="KT")

                def kproj(hp):
                    stage = psg.tile([P, S], F8, tag="kstg", name="kstg")
                    for rt in range(NRT_K):
                        ps = psS.tile([P, KG, 512], F32, tag="ss", name="ss")[
                            :, 0, :
                        ]
                        for dt in range(NDT):
                            nc.tensor.matmul(
                                ps,
                                wk[dt][:, :, hp * P : (hp + 1) * P],
                                xk[dt][:, :, rt * 512 : (rt + 1) * 512],
                                start=(dt == 0),
                                stop=False,
                                perf_mode=DR,
                            )
                        # bias via a 1-row accumulation matmul so the stage
                        # copy below needs no per-partition bias operand
                        nc.tensor.matmul(
                            ps,
                            bkr_t[:, hp, :],
                            ones_bf,
                            start=False,
                            stop=True,
                        )
                        nc.scalar.activation(
                            stage[:, rt * 512 : (rt + 1) * 512],
                            ps,
                            AF.Copy,
                        )
                    for h01 in range(2):
                        h = 2 * hp + h01
                        for i in range(2):
                            pb = h01 * DK + i * 32
                            (nc.sync if i == 0 else nc.gpsimd).dma_start(
                                KT[(h % 2) * 64 : (h % 2) * 64 + 32, h // 2, i, :],
                                stage[pb : pb + 32, :],
                            )

                def attn2(hp):
                    """Both heads of a pair per key-chunk: one score tile
                    [128, 2(head), 512] -> one exp op -> two pv matmuls,
                    deferred 3 steps behind their exps across the whole
                    (qt, kt) stream so the PE queue never parks."""
                    xo_t = XO[hp]
                    pvs_by_qt = {}
                    pending = []

                    def tail(qt, h01):
                        pv = pvs_by_qt[qt][h01]
                        rc64 = pdr.tile([DK, 512], F32, tag="rc", name="rc")
                        nc.vector.reciprocal(rc64, pv[DK : 2 * DK, :])
                        nc.vector.tensor_tensor(
                            xo_t[:, h01, qt * 512 : (qt + 1) * 512],
                            pv[0:DK, :],
                            rc64,
                            op=ALU.mult,
                        )

                    def pv_mms(qt, kt, ex):
                        for h01 in range(2):
                            nc.tensor.matmul(
                                pvs_by_qt[qt][h01],
                                Vt[kt][:, 2 * hp + h01, :, :],
                                ex[:, h01, :].bitcast(BF16),
                                start=(kt == 0),
                                stop=(kt == NKT - 1),
                            )
                        if kt == NKT - 1:
                            tail(qt, 0)
                            tail(qt, 1)

                    for qt in range(NQT):
                        pvs_by_qt[qt] = [
                            psPV.tile([2 * DK, 512], F32, tag="pv", name="pv")
                            for _ in range(2)
                        ]
                        for kt in range(NKT):
                            ss = psS.tile([P, KG, 512], F32, tag="ss", name="ss")
                            for h01 in range(2):
                                kb = h01 * 64
                                nc.tensor.matmul(
                                    ss[:, h01, :],
                                    KT[kb : kb + 32, hp, :, kt * P : (kt + 1) * P],
                                    QT[
                                        kb : kb + 32,
                                        hp,
                                        :,
                                        qt * 512 : (qt + 1) * 512,
                                    ],
                                    start=True,
                                    stop=True,
                                    perf_mode=DR,
                                )
                            ex = pde.tile([P, KG, 512], I16, tag="ex", name="ex")
                            if kt % 16 in (1, 3, 5, 7, 9, 11, 13, 15):
                                nc.vector.tensor_scalar(
                                    ex,
                                    ss,
                                    EXP_A,
                                    EXP_B,
                                    op0=ALU.mult,
                                    op1=ALU.add,
                                )
                            else:
                                nc.scalar.activation(
                                    ex[:, :, :].bitcast(BF16),
                                    ss,
                                    AF.Exp,
                                    scale=EXP_SCALE,
                                )
                            pending.append((qt, kt, ex))
                            if len(pending) > 3:
                                pv_mms(*pending.pop(0))
                    for it in pending:
                        pv_mms(*it)

                kproj(0)
                for hp in range(NHP):
                    if hp + 1 < NHP:
                        kproj(hp + 1)
                    attn2(hp)

            pdx_cm.__exit__(None, None, None)
            psg_cm.__exit__(None, None, None)

        # ---- Phase E: out = LN(x_o @ Wo^T + bo + q)  (bo pre-added to qres)
        with (
            tc.tile_pool(name="peq", bufs=4) as peq,
            tc.tile_pool(name="pey", bufs=4) as pey,
            tc.tile_pool(name="pst", bufs=8) as pst,
            tc.tile_pool(name="psE", bufs=6, space="PSUM") as psE,
        ):
            for rt in range(NRT_O):
                qr = peq.tile([P, D], BF16)
                nc.sync.dma_start(qr, qres[rt * P : (rt + 1) * P, :])
                y = pey.tile([P, D], F32)
                for o2 in range(2):
                    ps = psE.tile([P, 512], F32)
                    for hp in range(NOT):
                        nc.tensor.matmul(
                            ps,
                            XO[hp][:, :, rt * P : (rt + 1) * P],
                            wo[hp][:, :, o2 * 512 : (o2 + 1) * 512],
                            start=(hp == 0),
                            stop=(hp == NOT - 1),
                            perf_mode=DR,
                        )
                    nc.scalar.activation(
                        y[:, o2 * 512 : (o2 + 1) * 512],
                        ps,
                        AF.Copy,
                        scale=1.0 / 2048.0,
                    )
                    aeng = nc.vector if o2 == 0 else nc.gpsimd
                    aeng.tensor_tensor(
                        y[:, o2 * 512 : (o2 + 1) * 512],
                        y[:, o2 * 512 : (o2 + 1) * 512],
                        qr[:, o2 * 512 : (o2 + 1) * 512],
                        op=ALU.add,
                    )
                stats = pst.tile([P, 2, 6], F32)
                for sg in range(2):
                    nc.vector.bn_stats(
                        stats[:, sg, :], y[:, sg * 512 : (sg + 1) * 512]
                    )
                mv = pst.tile([P, 2], F32)
                nc.vector.bn_aggr(mv, stats)
                std = pst.tile([P, 1], F32)
                nc.scalar.activation(std, mv[:, 1:2], AF.Sqrt, bias=eps_t)
                rstd = pst.tile([P, 1], F32)
                nc.vector.reciprocal(rstd, std)
                nc.gpsimd.tensor_scalar(
                    y,
                    y,
                    mv[:, 0:1],
                    rstd,
                    op0=ALU.subtract,
                    op1=ALU.mult,
                )
                eng = nc.vector if rt % 2 == 0 else nc.gpsimd
                eng.tensor_mul(y, y, g_b)
                eng.tensor_add(y, y, b_b)
                nc.sync.dma_start(out[rt * P : (rt + 1) * P, :], y)
        pec_cm.__exit__(None, None, None)
        pwo_cm.__exit__(None, None, None)
        pxo_cm.__exit__(None, None, None)
        loop_cm.__exit__(None, None, None)
    _split_sync_waits(nc)
    return nc


_NC = None


def _get_nc():
    global _NC
    if _NC is None:
        _NC = build_nc()
    return _NC


def _split_dk(a):
    """[D, N] -> [64, NDT, 2, N] with d = dt*128 + i*64 + p."""
    Dd, N = a.shape
    return np.ascontiguousarray(
        a.reshape(NDT, 2, DK, N).transpose(2, 0, 1, 3)
    )


def prepare_in_maps(q, k, v, Wq, bq, Wk, bk, Wv, bv, Wo, bo, ln_g, ln_b):
    f = np.float32
    f8 = ml_dtypes.float8_e4m3
    q = np.asarray(q, f)
    k = np.asarray(k, f)
    v = np.asarray(v, f)
    wq8 = _split_dk(np.asarray(Wq, f).T * 32.0).astype(f8)
    wk8 = _split_dk(np.asarray(Wk, f).T * 32.0).astype(f8)
    wv8 = _split_dk(np.asarray(Wv, f).T * 32.0).astype(f8)
    wo8 = _split_dk(np.asarray(Wo, f).T * 32.0).astype(f8)
    common = {
        "wq8": wq8,
        "wk8": wk8,
        "wv8": wv8,
        "wo8": wo8,
        "bq32": np.ascontiguousarray((np.asarray(bq, f) * 32.0).reshape(NOT, P).T),
        "bk32": np.ascontiguousarray((np.asarray(bk, f) * 32.0).reshape(NOT, P).T),
        "bkr": (np.asarray(bk, f) * 32.0).reshape(NHP, P).astype(ml_dtypes.bfloat16),
        "bvb": (np.asarray(bv, f) * 32.0).astype(ml_dtypes.bfloat16),
        "ln_g": np.asarray(ln_g, f),
        "ln_b": np.asarray(ln_b, f),
    }
    in_maps = []
    for c in range(8):
        b_, half = divmod(c, 2)
        qs = q[b_, half * M : (half + 1) * M, :]
        qres_c = (qs + np.asarray(bo, f)[None, :]).astype(ml_dtypes.bfloat16)
        in_maps.append(
            dict(
                common,
                xq8=_split_dk(qs.T).astype(f8),
                xk8=_split_dk(k[b_].T).astype(f8),
                xv8=_split_dk(v[b_].T).astype(f8),
                qres=np.ascontiguousarray(qres_c),
            )
        )
    return in_maps


def kernel(q, k, v, Wq, bq, Wk, bk, Wv, bv, Wo, bo, ln_g, ln_b):
    nc = _get_nc()
    in_maps = prepare_in_maps(q, k, v, Wq, bq, Wk, bk, Wv, bv, Wo, bo, ln_g, ln_b)
    res = run_bass_kernel_spmd(nc, in_maps, core_ids=list(range(8)))
    out = np.empty((B, S, D), np.float32)
    for c in range(8):
        b_, half = divmod(c, 2)
        out[b_, half * M : (half + 1) * M, :] = res.results[c]["out"]
    return out


# revision 59
# speedup vs baseline: 1.4259x; 1.4259x over previous
"""MultiHeadedAttention block (B=4, S=2048, D=1024, H=16) on 8 TRN2 cores.

Sharding: core c handles batch b=c//2 and query-row half c%2 (1024 rows).
Each core computes full K/V projections for its batch (2x redundant within a
batch pair), attention for all 16 heads over its 1024 query rows, then
O-projection + residual + LayerNorm. No collectives.

All four projections and the QK^T scores run in fp8e4m3 with the DoubleRow
perf mode (2x PE rate). Scale management: weights are stored as W.T*32 in
fp8, activations x in fp8, so Q'=K'=32(xW+b) (stored fp8 in a split-dk
[32p, 2, .] layout for DoubleRow scores), V'=32(xWv+bv) (bf16). Raw scores
are 1024*(QK^T); the softmax exp applies scale 1/8192 = 1/(1024*sqrt(64)).
The ones-column of V gives the softmax denominator D via the PV matmul; the
reciprocal is broadcast with a ones(=2.0) stationary vector so
xo = pv * (2/D) = 64*(attn_out + bv) in fp8, and phase E folds the
1/(64*32) into a 1/2048 multiply before the residual add + LayerNorm.

The softmax exp is split across three engines: exact Exp on the Activation
engine plus the int16-bitcast approximation exp(x) ~= bitcast_bf16(
int16(x*128*log2e + 127*128 - 5.5)) on Pool and DVE (max ~3% weight error,
negligible through the diffuse softmax at this tolerance).
"""

import sys

if "/opt/trn_rl_repo" not in sys.path:
    sys.path.insert(0, "/opt/trn_rl_repo")

import ml_dtypes
import numpy as np

import concourse.bass as bass
import concourse.mybir as mybir
import concourse.tile as tile
from concourse.bass_utils import run_bass_kernel_spmd

B, S, D, H, DK = 4, 2048, 1024, 16, 64
P = 128
M = S // 2          # query rows per core
NDT = D // P        # 8 contraction chunks of 128
NOT = D // P        # 8 output-feature chunks (= head pairs)
NHP = H // 2        # 8 head pairs
NKT = S // P        # 16 key chunks of 128
NQT = M // 512      # 2 query 512-chunks
NRT_K = S // 512    # 4 key-row 512-chunks
NRT_V = S // P      # 16 V row chunks
NRT_O = M // P      # 8 output row chunks
KG = 2              # k-chunks per exp group
NKG = NKT // KG     # 8 exp groups per (head, qt)
F32 = mybir.dt.float32
F8 = mybir.dt.float8e4
BF16 = mybir.dt.bfloat16
I16 = mybir.dt.int16
MM_DT = mybir.dt.float32r
AF = mybir.ActivationFunctionType
ALU = mybir.AluOpType
DR = mybir.MatmulPerfMode.DoubleRow

LOG2E = 1.4426950408889634
EXP_SCALE = 1.0 / 8192.0                 # 1/(32*32*sqrt(DK))
EXP_A = 128.0 * LOG2E * EXP_SCALE        # int16-bitcast exp multiplier
EXP_B = 127.0 * 128.0 - 5.5              # exponent bias - mean sawtooth corr


def _split_sync_waits(nc, max_waits=1):
    """Split instructions carrying more than max_waits sem waits.

    The container's walrus rejects instructions with multiple sync wait
    commands, so excess waits move onto NoOp instructions inserted just
    before, on the same engine.
    """
    idx = 0
    for f in nc.m.functions:
        for blk in f.blocks:
            newl = []
            for inst in blk.instructions:
                si = inst.sync_info
                waits = list(si.on_wait) if si is not None and si.on_wait else []
                if len(waits) > max_waits:
                    extra = waits[max_waits:]
                    si.on_wait = waits[:max_waits]
                    for j in range(0, len(extra), max_waits):
                        nop = mybir.InstNoOp(name=f"I-wsplit-{idx}", ins=[], outs=[])
                        idx += 1
                        nop.engine = inst.engine
                        nop.sync_info = mybir.SyncInfo(
                            on_wait=extra[j : j + max_waits], on_update=[]
                        )
                        newl.append(nop)
                newl.append(inst)
            blk.instructions = newl


def build_nc(loops=0):
    nc = bass.Bass()
    xq8 = nc.dram_tensor("xq8", [DK, NDT, 2, M], F8, kind="ExternalInput")
    xk8 = nc.dram_tensor("xk8", [DK, NDT, 2, S], F8, kind="ExternalInput")
    xv8 = nc.dram_tensor("xv8", [DK, NDT, 2, S], F8, kind="ExternalInput")
    wq8 = nc.dram_tensor("wq8", [DK, NDT, 2, D], F8, kind="ExternalInput")
    wk8 = nc.dram_tensor("wk8", [DK, NDT, 2, D], F8, kind="ExternalInput")
    wv8 = nc.dram_tensor("wv8", [DK, NDT, 2, D], F8, kind="ExternalInput")
    wo8 = nc.dram_tensor("wo8", [DK, NDT, 2, D], F8, kind="ExternalInput")
    bq32 = nc.dram_tensor("bq32", [P, NOT], F32, kind="ExternalInput")
    bk32 = nc.dram_tensor("bk32", [P, NOT], F32, kind="ExternalInput")
    bkr = nc.dram_tensor("bkr", [NHP, P], BF16, kind="ExternalInput")
    bvb = nc.dram_tensor("bvb", [D], BF16, kind="ExternalInput")
    qres = nc.dram_tensor("qres", [M, D], BF16, kind="ExternalInput")
    gv = nc.dram_tensor("ln_g", [D], F32, kind="ExternalInput")
    bv2 = nc.dram_tensor("ln_b", [D], F32, kind="ExternalInput")
    gvb = nc.dram_tensor("gvb", [D], BF16, kind="ExternalInput")
    lnbb = nc.dram_tensor("lnbb", [D], BF16, kind="ExternalInput")
    out = nc.dram_tensor("out", [M, D], F32, kind="ExternalOutput")

    import contextlib

    with tile.TileContext(nc) as tc:
        loop_cm = tc.For_i(0, loops, 1) if loops else contextlib.nullcontext()
        loop_cm.__enter__()
        pxo_cm = tc.tile_pool(name="pxo", bufs=1)
        pxo = pxo_cm.__enter__()
        with (
            tc.tile_pool(name="pqv", bufs=1) as pqv,
        ):
            # attention outputs, fp8 split-head-pair layout for O-proj DR
            XO = [
                pxo.tile([DK, 2, M], F8, tag=f"XO{i}", name=f"XO{i}")
                for i in range(NHP)
            ]

            # Q' fp8, 2 heads across partitions: head h at partition
            # (h%2)*64, slot h//2, sub i = dk//32
            QT = pqv.tile([P, 8, 2, M], F8, tag="QT", name="QT")
            bq_p = pqv.tile([P, NOT], F32)
            bkr_t = pqv.tile([1, NHP, P], BF16)
            ones_bf = pqv.tile([1, 512], BF16)
            nc.vector.memset(ones_bf, 1.0)
            nc.sync.dma_start(bkr_t, bkr[:, :].rearrange("a p -> (a p)").partition_broadcast(1))
            bk_p = pqv.tile([P, NOT], F32)
            bv_b = pqv.tile([P, D], BF16)

            # Vt: [P, 2, H, DK] fp8; sub 1 is a 0.5-constant block so the
            # PV matmul replicates den/2 across output partitions 64..127
            Vt = []
            for rt in range(NRT_V):
                t = pqv.tile([P, H, 2, DK], F8, tag=f"Vt{rt}", name=f"Vt{rt}")
                (nc.vector if rt % 2 == 0 else nc.gpsimd).memset(t[:, :, 1, :], 0.5)
                Vt.append(t)

            # wv/xv load early so phase B starts without a DMA stall
            pwv_cm = tc.tile_pool(name="pwv", bufs=NDT // 2, side="right")
            pwv = pwv_cm.__enter__()
            wvp = [
                pwv.tile([DK, 2, 2, D], F8, tag="wv", name=f"wv{dp}")
                for dp in range(NDT // 2)
            ]
            wv = [wvp[dt // 2][:, dt % 2] for dt in range(NDT)]

            pbx_cm = tc.tile_pool(name="pbx", bufs=NDT // 2, side="right")
            pbx = pbx_cm.__enter__()
            xvp = [
                pbx.tile([DK, 2, 2, S], F8, tag="xv", name=f"xv{dp}")
                for dp in range(NDT // 2)
            ]
            xv = [xvp[dt // 2][:, dt % 2] for dt in range(NDT)]

            psAB_cm = tc.tile_pool(name="psAB", bufs=8, space="PSUM")
            psAB = psAB_cm.__enter__()
            psg_cm = tc.tile_pool(name="psg", bufs=3)
            psg = psg_cm.__enter__()

            # ---- Phase A: Q' = 32*(Wq @ x_q^T) + 32 bq, fp8 DR
            with (
                tc.tile_pool(name="pa", bufs=NDT // 2) as pa,
            ):
                wqp = []
                xqp = []
                for dp in range(NDT // 2):
                    wt = pa.tile([DK, 2, 2, D], F8, tag="wq", name=f"wq{dp}")
                    nc.sync.dma_start(wt, wq8[:, 2 * dp : 2 * dp + 2, :, :])
                    wqp.append(wt)
                    xt = pa.tile([DK, 2, 2, M], F8, tag="xq", name=f"xq{dp}")
                    nc.sync.dma_start(xt, xq8[:, 2 * dp : 2 * dp + 2, :, :])
                    xqp.append(xt)
                wq = [wqp[dt // 2][:, dt % 2] for dt in range(NDT)]
                xq = [xqp[dt // 2][:, dt % 2] for dt in range(NDT)]
                # behind phase A's own loads in the HWDGE queue: biases,
                # then the wv/xv pairs phase B consumes in dt order
                nc.sync.dma_start(bq_p, bq32[:, :])
                nc.sync.dma_start(bk_p, bk32[:, :])
                nc.sync.dma_start(bv_b, bvb[:].partition_broadcast(P))
                for dp in range(NDT // 2):
                    nc.sync.dma_start(wvp[dp], wv8[:, 2 * dp : 2 * dp + 2, :, :])
                    nc.sync.dma_start(xvp[dp], xv8[:, 2 * dp : 2 * dp + 2, :, :])
                for ot in range(NOT):
                    stage = psg.tile([P, M], F8, tag="qstg", name="qstg")
                    for qt in range(NQT):
                        ps = psAB.tile([P, 512], F32, tag="ps", name="ps")
                        for dt in range(NDT):
                            nc.tensor.matmul(
                                ps,
                                wq[dt][:, :, ot * P : (ot + 1) * P],
                                xq[dt][:, :, qt * 512 : (qt + 1) * 512],
                                start=(dt == 0),
                                stop=(dt == NDT - 1),
                                perf_mode=DR,
                            )
                        nc.vector.tensor_scalar_add(
                            stage[:, qt * 512 : (qt + 1) * 512],
                            ps,
                            bq_p[:, ot : ot + 1],
                        )
                    # repack the whole ot row into the split-dk
                    # 2-heads-across-partitions layout via 4 DMAs
                    for h01 in range(2):
                        h = 2 * ot + h01
                        for i in range(2):
                            pb = h01 * DK + i * 32
                            (nc.sync if i == 0 else nc.gpsimd).dma_start(
                                QT[(h % 2) * 64 : (h % 2) * 64 + 32, h // 2, i, :],
                                stage[pb : pb + 32, :],
                            )

            # xk/wk load during phase B so phase D starts without a DMA stall
            pdx_cm = tc.tile_pool(name="pdx", bufs=NDT // 2)
            pdx = pdx_cm.__enter__()
            xkp = []
            wkp = []
            for dp in range(NDT // 2):
                xt = pdx.tile([DK, 2, 2, S], F8, tag="xk", name=f"xk{dp}")
                nc.sync.dma_start(xt, xk8[:, 2 * dp : 2 * dp + 2, :, :])
                xkp.append(xt)
                wt = pdx.tile([DK, 2, 2, D], F8, tag="wk", name=f"wk{dp}")
                nc.sync.dma_start(wt, wk8[:, 2 * dp : 2 * dp + 2, :, :])
                wkp.append(wt)
            xk = [xkp[dt // 2][:, dt % 2] for dt in range(NDT)]
            wk = [wkp[dt // 2][:, dt % 2] for dt in range(NDT)]

            # ---- Phase B: V' = 32*(x_v @ Wv^T + bv), fp8 DR, bf16 out
            for rt in range(NRT_V):
                for o2 in range(2):
                    ps = psAB.tile([P, 512], F32, tag="ps", name="ps")
                    for dt in range(NDT):
                        nc.tensor.matmul(
                            ps,
                            xv[dt][:, :, rt * P : (rt + 1) * P],
                            wv[dt][:, :, o2 * 512 : (o2 + 1) * 512],
                            start=(dt == 0),
                            stop=(dt == NDT - 1),
                            perf_mode=DR,
                        )
                    nc.vector.tensor_tensor(
                        Vt[rt][:, o2 * 8 : (o2 + 1) * 8, 0, :],
                        ps[:, :].rearrange("p (h e) -> p h e", e=DK),
                        bv_b[:, o2 * 512 : (o2 + 1) * 512].rearrange(
                            "p (h e) -> p h e", e=DK
                        ),
                        op=ALU.add,
                    )

            pbx_cm.__exit__(None, None, None)
            pwv_cm.__exit__(None, None, None)
            psAB_cm.__exit__(None, None, None)

            # wo prefetch during D so phase E starts without a DMA stall
            pwo_cm = tc.tile_pool(name="pwo", bufs=NDT, side="right")
            pwo = pwo_cm.__enter__()
            wo = []
            for dt in range(NDT):
                t = pwo.tile([DK, 2, D], F8, tag="wo", name=f"wo{dt}")
                nc.sync.dma_start(t, wo8[:, dt, :, :])
                wo.append(t)
            pec_cm = tc.tile_pool(name="pec", bufs=1, side="right")
            pec = pec_cm.__enter__()
            g_b = pec.tile([P, D], BF16)
            b_b = pec.tile([P, D], BF16)
            eps_t = pec.tile([P, 1], F32)
            nc.sync.dma_start(g_b, gvb[:].partition_broadcast(P))
            nc.sync.dma_start(b_b, lnbb[:].partition_broadcast(P))
            nc.vector.memset(eps_t, 1e-5)

            # ---- Phase D: K' projection fused with attention
            with (
                tc.tile_pool(name="pdkt", bufs=1) as pdkt,
                tc.tile_pool(name="pde", bufs=6) as pde,
                tc.tile_pool(name="pdr", bufs=4) as pdr,
                tc.tile_pool(name="psS", bufs=3, space="PSUM") as psS,
                tc.tile_pool(name="psPV", bufs=2, space="PSUM") as psPV,
            ):
                # K' fp8, same 2-heads-across-partitions layout as QT
                KT = pdkt.tile([P, 8, 2, S], F8, tag="KT", name="KT")

                def kproj(hp):
                    stage = psg.tile([P, S], F8, tag="kstg", name="kstg")
                    for rt in range(NRT_K):
                        ps = psS.tile([P, KG, 512], F32, tag="ss", name="ss")[
                            :, 0, :
                        ]
                        for dt in range(NDT):
                            nc.tensor.matmul(
                                ps,
                                wk[dt][:, :, hp * P : (hp + 1) * P],
                                xk[dt][:, :, rt * 512 : (rt + 1) * 512],
                                start=(dt == 0),
                                stop=False,
                                perf_mode=DR,
                            )
                        # bias via a 1-row accumulation matmul so the stage
                        # copy below needs no per-partition bias operand
                        nc.tensor.matmul(
                            ps,
                            bkr_t[:, hp, :],
                            ones_bf,
                            start=False,
                            stop=True,
                        )
                        nc.scalar.activation(
                            stage[:, rt * 512 : (rt + 1) * 512],
                            ps,
                            AF.Copy,
                        )
                    for h01 in range(2):
                        h = 2 * hp + h01
                        for i in range(2):
                            pb = h01 * DK + i * 32
                            (nc.sync if i == 0 else nc.gpsimd).dma_start(
                                KT[(h % 2) * 64 : (h % 2) * 64 + 32, h // 2, i, :],
                                stage[pb : pb + 32, :],
                            )

                def attn2(hp):
                    """Both heads of a pair per key-chunk: one score tile
                    [128, 2(head), 512] -> one exp op -> two pv matmuls,
                    deferred 3 steps behind their exps across the whole
                    (qt, kt) stream so the PE queue never parks."""
                    xo_t = XO[hp]
                    pvs_by_qt = {}
                    pending = []

                    def tail(qt, h01):
                        pv = pvs_by_qt[qt][h01]
                        rc64 = pdr.tile([DK, 512], F32, tag="rc", name="rc")
                        nc.vector.reciprocal(rc64, pv[DK : 2 * DK, :])
                        nc.vector.tensor_tensor(
                            xo_t[:, h01, qt * 512 : (qt + 1) * 512],
                            pv[0:DK, :],
                            rc64,
                            op=ALU.mult,
                        )

                    def pv_mms(qt, kt, ex):
                        for h01 in range(2):
                            nc.tensor.matmul(
                                pvs_by_qt[qt][h01],
                                Vt[kt][:, 2 * hp + h01, :, :],
                                ex[:, h01, :].bitcast(BF16),
                                start=(kt == 0),
                                stop=(kt == NKT - 1),
                            )
                        if kt == NKT - 1:
                            tail(qt, 0)
                            tail(qt, 1)

                    for qt in range(NQT):
                        pvs_by_qt[qt] = [
                            psPV.tile([2 * DK, 512], F32, tag="pv", name="pv")
                            for _ in range(2)
                        ]
                        for kt in range(NKT):
                            ss = psS.tile([P, KG, 512], F32, tag="ss", name="ss")
                            for h01 in range(2):
                                kb = h01 * 64
                                nc.tensor.matmul(
                                    ss[:, h01, :],
                                    KT[kb : kb + 32, hp, :, kt * P : (kt + 1) * P],
                                    QT[
                                        kb : kb + 32,
                                        hp,
                                        :,
                                        qt * 512 : (qt + 1) * 512,
                                    ],
                                    start=True,
                                    stop=True,
                                    perf_mode=DR,
                                )
                            ex = pde.tile([P, KG, 512], I16, tag="ex", name="ex")
                            if kt % 16 in (1, 3, 5, 7, 9, 11, 13, 15):
                                nc.vector.tensor_scalar(
                                    ex,
                                    ss,
                                    EXP_A,
                                    EXP_B,
                                    op0=ALU.mult,
                                    op1=ALU.add,
                                )
                            else:
                                nc.scalar.activation(
                                    ex[:, :, :].bitcast(BF16),
                                    ss,
                                    AF.Exp,
                                    scale=EXP_SCALE,
                                )
                            pending.append((qt, kt, ex))
                            if len(pending) > 3:
                                pv_mms(*pending.pop(0))
                    for it in pending:
                        pv_mms(*it)

                kproj(0)
                for hp in range(NHP):
                    if hp + 1 < NHP:
                        kproj(hp + 1)
                    attn2(hp)

            pdx_cm.__exit__(None, None, None)
            psg_cm.__exit__(None, None, None)

        # ---- Phase E: out = LN(x_o @ Wo^T + bo + q)  (bo pre-added to qres)
        with (
            tc.tile_pool(name="peq", bufs=4) as peq,
            tc.tile_pool(name="pey", bufs=4) as pey,
            tc.tile_pool(name="pst", bufs=8) as pst,
            tc.tile_pool(name="psE", bufs=6, space="PSUM") as psE,
        ):
            for rt in range(NRT_O):
                qr = peq.tile([P, D], BF16)
                nc.sync.dma_start(qr, qres[rt * P : (rt + 1) * P, :])
                y = pey.tile([P, D], BF16)
                for o2 in range(2):
                    ps = psE.tile([P, 512], F32)
                    for hp in range(NOT):
                        nc.tensor.matmul(
                            ps,
                            XO[hp][:, :, rt * P : (rt + 1) * P],
                            wo[hp][:, :, o2 * 512 : (o2 + 1) * 512],
                            start=(hp == 0),
                            stop=(hp == NOT - 1),
                            perf_mode=DR,
                        )
                    nc.scalar.activation(
                        y[:, o2 * 512 : (o2 + 1) * 512],
                        ps,
                        AF.Copy,
                        scale=1.0 / 2048.0,
                    )
                    aeng = nc.vector if o2 == 0 else nc.gpsimd
                    aeng.tensor_tensor(
                        y[:, o2 * 512 : (o2 + 1) * 512],
                        y[:, o2 * 512 : (o2 + 1) * 512],
                        qr[:, o2 * 512 : (o2 + 1) * 512],
                        op=ALU.add,
                    )
                stats = pst.tile([P, 2, 6], F32)
                for sg in range(2):
                    nc.vector.bn_stats(
                        stats[:, sg, :], y[:, sg * 512 : (sg + 1) * 512]
                    )
                mv = pst.tile([P, 2], F32)
                nc.vector.bn_aggr(mv, stats)
                std = pst.tile([P, 1], F32)
                nc.scalar.activation(std, mv[:, 1:2], AF.Sqrt, bias=eps_t)
                rstd = pst.tile([P, 1], F32)
                nc.vector.reciprocal(rstd, std)
                nc.gpsimd.tensor_scalar(
                    y,
                    y,
                    mv[:, 0:1],
                    rstd,
                    op0=ALU.subtract,
                    op1=ALU.mult,
                )
                y32 = pey.tile([P, D], F32, tag="y32", name="y32")
                eng = nc.vector if rt % 2 == 0 else nc.gpsimd
                eng.tensor_mul(y, y, g_b)
                eng.tensor_add(y32, y, b_b)
                nc.sync.dma_start(out[rt * P : (rt + 1) * P, :], y32)
        pec_cm.__exit__(None, None, None)
        pwo_cm.__exit__(None, None, None)
        pxo_cm.__exit__(None, None, None)
        loop_cm.__exit__(None, None, None)
    _split_sync_waits(nc)
    return nc


_NC = None


def _get_nc():
    global _NC
    if _NC is None:
        _NC = build_nc()
    return _NC


def _split_dk(a):
    """[D, N] -> [64, NDT, 2, N] with d = dt*128 + i*64 + p."""
    Dd, N = a.shape
    return np.ascontiguousarray(
        a.reshape(NDT, 2, DK, N).transpose(2, 0, 1, 3)
    )


def prepare_in_maps(q, k, v, Wq, bq, Wk, bk, Wv, bv, Wo, bo, ln_g, ln_b):
    f = np.float32
    f8 = ml_dtypes.float8_e4m3
    q = np.asarray(q, f)
    k = np.asarray(k, f)
    v = np.asarray(v, f)
    wq8 = _split_dk(np.asarray(Wq, f).T * 32.0).astype(f8)
    wk8 = _split_dk(np.asarray(Wk, f).T * 32.0).astype(f8)
    wv8 = _split_dk(np.asarray(Wv, f).T * 32.0).astype(f8)
    wo8 = _split_dk(np.asarray(Wo, f).T * 32.0).astype(f8)
    common = {
        "wq8": wq8,
        "wk8": wk8,
        "wv8": wv8,
        "wo8": wo8,
        "bq32": np.ascontiguousarray((np.asarray(bq, f) * 32.0).reshape(NOT, P).T),
        "bk32": np.ascontiguousarray((np.asarray(bk, f) * 32.0).reshape(NOT, P).T),
        "bkr": (np.asarray(bk, f) * 32.0).reshape(NHP, P).astype(ml_dtypes.bfloat16),
        "bvb": (np.asarray(bv, f) * 32.0).astype(ml_dtypes.bfloat16),
        "ln_g": np.asarray(ln_g, f),
        "ln_b": np.asarray(ln_b, f),
        "gvb": np.asarray(ln_g, f).astype(ml_dtypes.bfloat16),
        "lnbb": np.asarray(ln_b, f).astype(ml_dtypes.bfloat16),
    }
    in_maps = []
    for c in range(8):
        b_, half = divmod(c, 2)
        qs = q[b_, half * M : (half + 1) * M, :]
        qres_c = (qs + np.asarray(bo, f)[None, :]).astype(ml_dtypes.bfloat16)
        in_maps.append(
            dict(
                common,
                xq8=_split_dk(qs.T).astype(f8),
                xk8=_split_dk(k[b_].T).astype(f8),
                xv8=_split_dk(v[b_].T).astype(f8),
                qres=np.ascontiguousarray(qres_c),
            )
        )
    return in_maps


def kernel(q, k, v, Wq, bq, Wk, bk, Wv, bv, Wo, bo, ln_g, ln_b):
    nc = _get_nc()
    in_maps = prepare_in_maps(q, k, v, Wq, bq, Wk, bk, Wv, bv, Wo, bo, ln_g, ln_b)
    res = run_bass_kernel_spmd(nc, in_maps, core_ids=list(range(8)))
    out = np.empty((B, S, D), np.float32)
    for c in range(8):
        b_, half = divmod(c, 2)
        out[b_, half * M : (half + 1) * M, :] = res.results[c]["out"]
    return out


# revision 70
# speedup vs baseline: 1.5124x; 1.0607x over previous
"""MultiHeadedAttention block (B=4, S=2048, D=1024, H=16) on 8 TRN2 cores.

Sharding: core c handles batch b=c//2 and query-row half c%2 (1024 rows).
Each core computes full K/V projections for its batch (2x redundant within a
batch pair), attention for all 16 heads over its 1024 query rows, then
O-projection + residual + LayerNorm. No collectives.

All four projections and the QK^T scores run in fp8e4m3 with the DoubleRow
perf mode (2x PE rate). Scale management: weights are stored as W.T*32 in
fp8, activations x in fp8, so Q'=K'=32(xW+b) (stored fp8 in a split-dk
[32p, 2, .] layout for DoubleRow scores), V'=32(xWv+bv) (bf16). Raw scores
are 1024*(QK^T); the softmax exp applies scale 1/8192 = 1/(1024*sqrt(64)).
The ones-column of V gives the softmax denominator D via the PV matmul; the
reciprocal is broadcast with a ones(=2.0) stationary vector so
xo = pv * (2/D) = 64*(attn_out + bv) in fp8, and phase E folds the
1/(64*32) into a 1/2048 multiply before the residual add + LayerNorm.

The softmax exp is split across three engines: exact Exp on the Activation
engine plus the int16-bitcast approximation exp(x) ~= bitcast_bf16(
int16(x*128*log2e + 127*128 - 5.5)) on Pool and DVE (max ~3% weight error,
negligible through the diffuse softmax at this tolerance).
"""

import sys

if "/opt/trn_rl_repo" not in sys.path:
    sys.path.insert(0, "/opt/trn_rl_repo")

import ml_dtypes
import numpy as np

import concourse.bass as bass
import concourse.mybir as mybir
import concourse.tile as tile
from concourse.bass_utils import run_bass_kernel_spmd

B, S, D, H, DK = 4, 2048, 1024, 16, 64
P = 128
M = S // 2          # query rows per core
NDT = D // P        # 8 contraction chunks of 128
NOT = D // P        # 8 output-feature chunks (= head pairs)
NHP = H // 2        # 8 head pairs
NKT = S // P        # 16 key chunks of 128
NQT = M // 512      # 2 query 512-chunks
NRT_K = S // 512    # 4 key-row 512-chunks
NRT_V = S // P      # 16 V row chunks
NRT_O = M // P      # 8 output row chunks
KG = 2              # k-chunks per exp group
NKG = NKT // KG     # 8 exp groups per (head, qt)
F32 = mybir.dt.float32
F8 = mybir.dt.float8e4
BF16 = mybir.dt.bfloat16
I16 = mybir.dt.int16
MM_DT = mybir.dt.float32r
AF = mybir.ActivationFunctionType
ALU = mybir.AluOpType
DR = mybir.MatmulPerfMode.DoubleRow

LOG2E = 1.4426950408889634
EXP_SCALE = 1.0 / 8192.0                 # 1/(32*32*sqrt(DK))
EXP_A = 128.0 * LOG2E * EXP_SCALE        # int16-bitcast exp multiplier
EXP_B = 127.0 * 128.0 - 5.5              # exponent bias - mean sawtooth corr


def _split_sync_waits(nc, max_waits=1):
    """Split instructions carrying more than max_waits sem waits.

    The container's walrus rejects instructions with multiple sync wait
    commands, so excess waits move onto NoOp instructions inserted just
    before, on the same engine.
    """
    idx = 0
    for f in nc.m.functions:
        for blk in f.blocks:
            newl = []
            for inst in blk.instructions:
                si = inst.sync_info
                waits = list(si.on_wait) if si is not None and si.on_wait else []
                if len(waits) > max_waits:
                    extra = waits[max_waits:]
                    si.on_wait = waits[:max_waits]
                    for j in range(0, len(extra), max_waits):
                        nop = mybir.InstNoOp(name=f"I-wsplit-{idx}", ins=[], outs=[])
                        idx += 1
                        nop.engine = inst.engine
                        nop.sync_info = mybir.SyncInfo(
                            on_wait=extra[j : j + max_waits], on_update=[]
                        )
                        newl.append(nop)
                newl.append(inst)
            blk.instructions = newl


def build_nc(loops=0):
    nc = bass.Bass()
    xq8 = nc.dram_tensor("xq8", [DK, NDT, 2, M], F8, kind="ExternalInput")
    xk8 = nc.dram_tensor("xk8", [DK, NDT, 2, S], F8, kind="ExternalInput")
    xv8 = nc.dram_tensor("xv8", [DK, NDT, 2, S], F8, kind="ExternalInput")
    wq8 = nc.dram_tensor("wq8", [DK, NDT, 2, D], F8, kind="ExternalInput")
    wk8 = nc.dram_tensor("wk8", [DK, NDT, 2, D], F8, kind="ExternalInput")
    wv8 = nc.dram_tensor("wv8", [DK, NDT, 2, D], F8, kind="ExternalInput")
    wo8 = nc.dram_tensor("wo8", [DK, NDT, 2, D], F8, kind="ExternalInput")
    bq32 = nc.dram_tensor("bq32", [P, NOT], F32, kind="ExternalInput")
    bk32 = nc.dram_tensor("bk32", [P, NOT], F32, kind="ExternalInput")
    bkr = nc.dram_tensor("bkr", [NHP, P], BF16, kind="ExternalInput")
    bvb = nc.dram_tensor("bvb", [D], BF16, kind="ExternalInput")
    qres = nc.dram_tensor("qres", [M, D], BF16, kind="ExternalInput")
    gv = nc.dram_tensor("ln_g", [D], F32, kind="ExternalInput")
    bv2 = nc.dram_tensor("ln_b", [D], F32, kind="ExternalInput")
    gvb = nc.dram_tensor("gvb", [D], BF16, kind="ExternalInput")
    lnbb = nc.dram_tensor("lnbb", [D], BF16, kind="ExternalInput")
    out = nc.dram_tensor("out", [M, D], F32, kind="ExternalOutput")

    import contextlib

    with tile.TileContext(nc) as tc:
        loop_cm = tc.For_i(0, loops, 1) if loops else contextlib.nullcontext()
        loop_cm.__enter__()
        pxo_cm = tc.tile_pool(name="pxo", bufs=1)
        pxo = pxo_cm.__enter__()
        with (
            tc.tile_pool(name="pqv", bufs=1) as pqv,
        ):
            # attention outputs, fp8 split-head-pair layout for O-proj DR
            XO = [
                pxo.tile([DK, 2, M], F8, tag=f"XO{i}", name=f"XO{i}")
                for i in range(NHP)
            ]

            # Q' fp8, 2 heads across partitions: head h at partition
            # (h%2)*64, slot h//2, sub i = dk//32
            QT = pqv.tile([P, 8, 2, M], F8, tag="QT", name="QT")
            bq_p = pqv.tile([P, NOT], F32)
            bkr_t = pqv.tile([1, NHP, P], BF16)
            ones_bf = pqv.tile([1, 512], BF16)
            nc.vector.memset(ones_bf, 1.0)
            nc.sync.dma_start(bkr_t, bkr[:, :].rearrange("a p -> (a p)").partition_broadcast(1))
            bk_p = pqv.tile([P, NOT], F32)
            bv_b = pqv.tile([P, D], BF16)

            # Vt: [P, 2, H, DK] fp8; sub 1 is a 0.5-constant block so the
            # PV matmul replicates den/2 across output partitions 64..127
            Vt = []
            for rt in range(NRT_V):
                t = pqv.tile([P, H, 2, DK], F8, tag=f"Vt{rt}", name=f"Vt{rt}")
                (nc.vector if rt % 2 == 0 else nc.gpsimd).memset(t[:, :, 1, :], 0.5)
                Vt.append(t)

            # wv/xv load early so phase B starts without a DMA stall
            pwv_cm = tc.tile_pool(name="pwv", bufs=NDT // 2, side="right")
            pwv = pwv_cm.__enter__()
            wvp = [
                pwv.tile([DK, 2, 2, D], F8, tag="wv", name=f"wv{dp}")
                for dp in range(NDT // 2)
            ]
            wv = [wvp[dt // 2][:, dt % 2] for dt in range(NDT)]

            pbx_cm = tc.tile_pool(name="pbx", bufs=NDT // 2, side="right")
            pbx = pbx_cm.__enter__()
            xvp = [
                pbx.tile([DK, 2, 2, S], F8, tag="xv", name=f"xv{dp}")
                for dp in range(NDT // 2)
            ]
            xv = [xvp[dt // 2][:, dt % 2] for dt in range(NDT)]

            psAB_cm = tc.tile_pool(name="psAB", bufs=8, space="PSUM")
            psAB = psAB_cm.__enter__()
            psg_cm = tc.tile_pool(name="psg", bufs=3)
            psg = psg_cm.__enter__()

            # ---- Phase A: Q' = 32*(Wq @ x_q^T) + 32 bq, fp8 DR
            with (
                tc.tile_pool(name="pa", bufs=NDT // 2) as pa,
            ):
                wqp = []
                xqp = []
                for dp in range(NDT // 2):
                    wt = pa.tile([DK, 2, 2, D], F8, tag="wq", name=f"wq{dp}")
                    nc.sync.dma_start(wt, wq8[:, 2 * dp : 2 * dp + 2, :, :])
                    wqp.append(wt)
                    xt = pa.tile([DK, 2, 2, M], F8, tag="xq", name=f"xq{dp}")
                    nc.sync.dma_start(xt, xq8[:, 2 * dp : 2 * dp + 2, :, :])
                    xqp.append(xt)
                wq = [wqp[dt // 2][:, dt % 2] for dt in range(NDT)]
                xq = [xqp[dt // 2][:, dt % 2] for dt in range(NDT)]
                # behind phase A's own loads in the HWDGE queue: biases,
                # then the wv/xv pairs phase B consumes in dt order
                nc.sync.dma_start(bq_p, bq32[:, :])
                nc.sync.dma_start(bk_p, bk32[:, :])
                nc.sync.dma_start(bv_b, bvb[:].partition_broadcast(P))
                for dp in range(NDT // 2):
                    nc.sync.dma_start(wvp[dp], wv8[:, 2 * dp : 2 * dp + 2, :, :])
                    nc.sync.dma_start(xvp[dp], xv8[:, 2 * dp : 2 * dp + 2, :, :])
                for ot in range(NOT):
                    stage = psg.tile([P, M], F8, tag="qstg", name="qstg")
                    for qt in range(NQT):
                        ps = psAB.tile([P, 512], F32, tag="ps", name="ps")
                        for dt in range(NDT):
                            nc.tensor.matmul(
                                ps,
                                wq[dt][:, :, ot * P : (ot + 1) * P],
                                xq[dt][:, :, qt * 512 : (qt + 1) * 512],
                                start=(dt == 0),
                                stop=(dt == NDT - 1),
                                perf_mode=DR,
                            )
                        nc.vector.tensor_scalar_add(
                            stage[:, qt * 512 : (qt + 1) * 512],
                            ps,
                            bq_p[:, ot : ot + 1],
                        )
                    # repack the whole ot row into the split-dk
                    # 2-heads-across-partitions layout via 4 DMAs
                    for h01 in range(2):
                        h = 2 * ot + h01
                        for i in range(2):
                            pb = h01 * DK + i * 32
                            (nc.sync if i == 0 else nc.gpsimd).dma_start(
                                QT[(h % 2) * 64 : (h % 2) * 64 + 32, h // 2, i, :],
                                stage[pb : pb + 32, :],
                            )

            # xk/wk load during phase B so phase D starts without a DMA stall
            pdx_cm = tc.tile_pool(name="pdx", bufs=NDT // 2)
            pdx = pdx_cm.__enter__()
            xkp = []
            wkp = []
            for dp in range(NDT // 2):
                xt = pdx.tile([DK, 2, 2, S], F8, tag="xk", name=f"xk{dp}")
                nc.sync.dma_start(xt, xk8[:, 2 * dp : 2 * dp + 2, :, :])
                xkp.append(xt)
                wt = pdx.tile([DK, 2, 2, D], F8, tag="wk", name=f"wk{dp}")
                nc.sync.dma_start(wt, wk8[:, 2 * dp : 2 * dp + 2, :, :])
                wkp.append(wt)
            xk = [xkp[dt // 2][:, dt % 2] for dt in range(NDT)]
            wk = [wkp[dt // 2][:, dt % 2] for dt in range(NDT)]

            # ---- Phase B: V' = 32*(x_v @ Wv^T + bv), fp8 DR, bf16 out
            for rt in range(NRT_V):
                for o2 in range(2):
                    ps = psAB.tile([P, 512], F32, tag="ps", name="ps")
                    for dt in range(NDT):
                        nc.tensor.matmul(
                            ps,
                            xv[dt][:, :, rt * P : (rt + 1) * P],
                            wv[dt][:, :, o2 * 512 : (o2 + 1) * 512],
                            start=(dt == 0),
                            stop=(dt == NDT - 1),
                            perf_mode=DR,
                        )
                    nc.vector.tensor_tensor(
                        Vt[rt][:, o2 * 8 : (o2 + 1) * 8, 0, :],
                        ps[:, :].rearrange("p (h e) -> p h e", e=DK),
                        bv_b[:, o2 * 512 : (o2 + 1) * 512].rearrange(
                            "p (h e) -> p h e", e=DK
                        ),
                        op=ALU.add,
                    )

            pbx_cm.__exit__(None, None, None)
            pwv_cm.__exit__(None, None, None)
            psAB_cm.__exit__(None, None, None)

            # wo prefetch during D so phase E starts without a DMA stall
            pwo_cm = tc.tile_pool(name="pwo", bufs=NDT, side="right")
            pwo = pwo_cm.__enter__()
            wo = []
            for dt in range(NDT):
                t = pwo.tile([DK, 2, D], F8, tag="wo", name=f"wo{dt}")
                nc.sync.dma_start(t, wo8[:, dt, :, :])
                wo.append(t)
            pec_cm = tc.tile_pool(name="pec", bufs=1, side="right")
            pec = pec_cm.__enter__()
            g_b = pec.tile([P, D], BF16)
            b_b = pec.tile([P, D], BF16)
            eps_t = pec.tile([P, 1], F32)
            nc.sync.dma_start(g_b, gvb[:].partition_broadcast(P))
            nc.sync.dma_start(b_b, lnbb[:].partition_broadcast(P))
            nc.vector.memset(eps_t, 1e-5)

            # ---- Phase D: K' projection fused with attention
            with (
                tc.tile_pool(name="pdkt", bufs=1) as pdkt,
                tc.tile_pool(name="pde", bufs=6) as pde,
                tc.tile_pool(name="pdr", bufs=4) as pdr,
                tc.tile_pool(name="psS", bufs=3, space="PSUM") as psS,
                tc.tile_pool(name="psPV", bufs=2, space="PSUM") as psPV,
            ):
                # K' fp8, same 2-heads-across-partitions layout as QT
                KT = pdkt.tile([P, 8, 2, S], F8, tag="KT", name="KT")

                def kproj(hp):
                    stage = psg.tile([P, S], F8, tag="kstg", name="kstg")
                    for rt in range(NRT_K):
                        ps = psS.tile([P, KG, 512], F32, tag="ss", name="ss")[
                            :, 0, :
                        ]
                        for dt in range(NDT):
                            nc.tensor.matmul(
                                ps,
                                wk[dt][:, :, hp * P : (hp + 1) * P],
                                xk[dt][:, :, rt * 512 : (rt + 1) * 512],
                                start=(dt == 0),
                                stop=False,
                                perf_mode=DR,
                            )
                        # bias via a 1-row accumulation matmul so the stage
                        # copy below needs no per-partition bias operand
                        nc.tensor.matmul(
                            ps,
                            bkr_t[:, hp, :],
                            ones_bf,
                            start=False,
                            stop=True,
                        )
                        nc.scalar.activation(
                            stage[:, rt * 512 : (rt + 1) * 512],
                            ps,
                            AF.Copy,
                        )
                    for h01 in range(2):
                        h = 2 * hp + h01
                        for i in range(2):
                            pb = h01 * DK + i * 32
                            (nc.sync if i == 0 else nc.gpsimd).dma_start(
                                KT[(h % 2) * 64 : (h % 2) * 64 + 32, h // 2, i, :],
                                stage[pb : pb + 32, :],
                            )

                def attn2(hp):
                    """Both heads of a pair per key-chunk: one score tile
                    [128, 2(head), 512] -> one exp op -> two pv matmuls,
                    deferred 3 steps behind their exps across the whole
                    (qt, kt) stream so the PE queue never parks."""
                    xo_t = XO[hp]
                    pvs_by_qt = {}
                    pending = []

                    def tail(qt, h01):
                        pv = pvs_by_qt[qt][h01]
                        pvsb = pdr.tile([2 * DK, 512], F32, tag="pvsb", name="pvsb")
                        nc.scalar.activation(pvsb, pv, AF.Copy)
                        rc64 = pdr.tile([DK, 512], F32, tag="rc", name="rc")
                        nc.vector.reciprocal(rc64, pvsb[DK : 2 * DK, :])
                        nc.gpsimd.tensor_tensor(
                            xo_t[:, h01, qt * 512 : (qt + 1) * 512],
                            pvsb[0:DK, :],
                            rc64,
                            op=ALU.mult,
                        )

                    def pv_mms(qt, kt, ex):
                        for h01 in range(2):
                            nc.tensor.matmul(
                                pvs_by_qt[qt][h01],
                                Vt[kt][:, 2 * hp + h01, :, :],
                                ex[:, h01, :].bitcast(BF16),
                                start=(kt == 0),
                                stop=(kt == NKT - 1),
                            )
                        if kt == NKT - 1:
                            tail(qt, 0)
                            tail(qt, 1)

                    for qt in range(NQT):
                        pvs_by_qt[qt] = [
                            psPV.tile([2 * DK, 512], F32, tag="pv", name="pv")
                            for _ in range(2)
                        ]
                        for kt in range(NKT):
                            ss = psS.tile([P, KG, 512], F32, tag="ss", name="ss")
                            for h01 in range(2):
                                kb = h01 * 64
                                nc.tensor.matmul(
                                    ss[:, h01, :],
                                    KT[kb : kb + 32, hp, :, kt * P : (kt + 1) * P],
                                    QT[
                                        kb : kb + 32,
                                        hp,
                                        :,
                                        qt * 512 : (qt + 1) * 512,
                                    ],
                                    start=True,
                                    stop=True,
                                    perf_mode=DR,
                                )
                            ex = pde.tile([P, KG, 512], I16, tag="ex", name="ex")
                            if kt % 16 in (1, 3, 5, 7, 9, 11, 13, 15):
                                nc.vector.tensor_scalar(
                                    ex,
                                    ss,
                                    EXP_A,
                                    EXP_B,
                                    op0=ALU.mult,
                                    op1=ALU.add,
                                )
                            else:
                                nc.scalar.activation(
                                    ex[:, :, :].bitcast(BF16),
                                    ss,
                                    AF.Exp,
                                    scale=EXP_SCALE,
                                )
                            pending.append((qt, kt, ex))
                            if len(pending) > 3:
                                pv_mms(*pending.pop(0))
                    for it in pending:
                        pv_mms(*it)

                kproj(0)
                for hp in range(NHP):
                    if hp + 1 < NHP:
                        kproj(hp + 1)
                    attn2(hp)

            pdx_cm.__exit__(None, None, None)
            psg_cm.__exit__(None, None, None)

        # ---- Phase E: out = LN(x_o @ Wo^T + bo + q)  (bo pre-added to qres)
        with (
            tc.tile_pool(name="peq", bufs=4) as peq,
            tc.tile_pool(name="pey", bufs=4) as pey,
            tc.tile_pool(name="pst", bufs=8) as pst,
            tc.tile_pool(name="psE", bufs=6, space="PSUM") as psE,
        ):
            for rt in range(NRT_O):
                qr = peq.tile([P, D], BF16)
                nc.sync.dma_start(qr, qres[rt * P : (rt + 1) * P, :])
                y = pey.tile([P, D], BF16)
                for o2 in range(2):
                    ps = psE.tile([P, 512], F32)
                    for hp in range(NOT):
                        nc.tensor.matmul(
                            ps,
                            XO[hp][:, :, rt * P : (rt + 1) * P],
                            wo[hp][:, :, o2 * 512 : (o2 + 1) * 512],
                            start=(hp == 0),
                            stop=(hp == NOT - 1),
                            perf_mode=DR,
                        )
                    nc.scalar.activation(
                        y[:, o2 * 512 : (o2 + 1) * 512],
                        ps,
                        AF.Copy,
                        scale=1.0 / 2048.0,
                    )
                    aeng = nc.vector
                    aeng.tensor_tensor(
                        y[:, o2 * 512 : (o2 + 1) * 512],
                        y[:, o2 * 512 : (o2 + 1) * 512],
                        qr[:, o2 * 512 : (o2 + 1) * 512],
                        op=ALU.add,
                    )
                stats = pst.tile([P, 2, 6], F32)
                for sg in range(2):
                    nc.vector.bn_stats(
                        stats[:, sg, :], y[:, sg * 512 : (sg + 1) * 512]
                    )
                mv = pst.tile([P, 2], F32)
                nc.vector.bn_aggr(mv, stats)
                std = pst.tile([P, 1], F32)
                nc.scalar.activation(std, mv[:, 1:2], AF.Sqrt, bias=eps_t)
                rstd = pst.tile([P, 1], F32)
                nc.vector.reciprocal(rstd, std)
                nc.vector.tensor_scalar(
                    y,
                    y,
                    mv[:, 0:1],
                    rstd,
                    op0=ALU.subtract,
                    op1=ALU.mult,
                )
                y32 = pey.tile([P, D], F32, tag="y32", name="y32")
                eng = nc.vector if rt % 2 == 0 else nc.gpsimd
                eng.tensor_mul(y, y, g_b)
                eng.tensor_add(y32, y, b_b)
                nc.sync.dma_start(out[rt * P : (rt + 1) * P, :], y32)
        pec_cm.__exit__(None, None, None)
        pwo_cm.__exit__(None, None, None)
        pxo_cm.__exit__(None, None, None)
        loop_cm.__exit__(None, None, None)
    _split_sync_waits(nc)
    return nc


_NC = None


def _get_nc():
    global _NC
    if _NC is None:
        _NC = build_nc()
    return _NC


def _split_dk(a):
    """[D, N] -> [64, NDT, 2, N] with d = dt*128 + i*64 + p."""
    Dd, N = a.shape
    return np.ascontiguousarray(
        a.reshape(NDT, 2, DK, N).transpose(2, 0, 1, 3)
    )


def prepare_in_maps(q, k, v, Wq, bq, Wk, bk, Wv, bv, Wo, bo, ln_g, ln_b):
    f = np.float32
    f8 = ml_dtypes.float8_e4m3
    q = np.asarray(q, f)
    k = np.asarray(k, f)
    v = np.asarray(v, f)
    wq8 = _split_dk(np.asarray(Wq, f).T * 32.0).astype(f8)
    wk8 = _split_dk(np.asarray(Wk, f).T * 32.0).astype(f8)
    wv8 = _split_dk(np.asarray(Wv, f).T * 32.0).astype(f8)
    wo8 = _split_dk(np.asarray(Wo, f).T * 32.0).astype(f8)
    common = {
        "wq8": wq8,
        "wk8": wk8,
        "wv8": wv8,
        "wo8": wo8,
        "bq32": np.ascontiguousarray((np.asarray(bq, f) * 32.0).reshape(NOT, P).T),
        "bk32": np.ascontiguousarray((np.asarray(bk, f) * 32.0).reshape(NOT, P).T),
        "bkr": (np.asarray(bk, f) * 32.0).reshape(NHP, P).astype(ml_dtypes.bfloat16),
        "bvb": (np.asarray(bv, f) * 32.0).astype(ml_dtypes.bfloat16),
        "ln_g": np.asarray(ln_g, f),
        "ln_b": np.asarray(ln_b, f),
        "gvb": np.asarray(ln_g, f).astype(ml_dtypes.bfloat16),
        "lnbb": np.asarray(ln_b, f).astype(ml_dtypes.bfloat16),
    }
    in_maps = []
    for c in range(8):
        b_, half = divmod(c, 2)
        qs = q[b_, half * M : (half + 1) * M, :]
        qres_c = (qs + np.asarray(bo, f)[None, :]).astype(ml_dtypes.bfloat16)
        in_maps.append(
            dict(
                common,
                xq8=_split_dk(qs.T).astype(f8),
                xk8=_split_dk(k[b_].T).astype(f8),
                xv8=_split_dk(v[b_].T).astype(f8),
                qres=np.ascontiguousarray(qres_c),
            )
        )
    return in_maps


def kernel(q, k, v, Wq, bq, Wk, bk, Wv, bv, Wo, bo, ln_g, ln_b):
    nc = _get_nc()
    in_maps = prepare_in_maps(q, k, v, Wq, bq, Wk, bk, Wv, bv, Wo, bo, ln_g, ln_b)
    res = run_bass_kernel_spmd(nc, in_maps, core_ids=list(range(8)))
    out = np.empty((B, S, D), np.float32)
    for c in range(8):
        b_, half = divmod(c, 2)
        out[b_, half * M : (half + 1) * M, :] = res.results[c]["out"]
    return out
